# revision 22
# baseline (speedup 1.0000x reference)
"""Memory-efficient multi-head cross-attention on 8 TRN2 NeuronCores.

Sharding: batch (2) x head-block (4 heads each) across 8 cores, tensor-parallel
qkv/o projections; an AllToAll inside each 4-core batch group converts the
head-sharded context into q-sharded full-width context so o-proj + residual +
LayerNorm run fully local on 512 query rows per core.

kernel(**inputs) takes the FULL unsharded inputs and returns the FULL output.
"""

import sys
import types

import ml_dtypes
import numpy as np

# ---------------------------------------------------------------------------
# Environment shims (must run before concourse imports are used)
# ---------------------------------------------------------------------------


def _install_ntff_shim():
    """Provide antenv.axon_hooks (absent in this image) so that
    run_bass_kernel_spmd(trace=True) can capture NTFF profiles via the
    axon ctypes hook. Harmless when tracing is off."""
    if "antenv.axon_hooks" in sys.modules:
        return
    hook = None
    try:
        from trn_agent_boot.trn_boot import _ntff_profile_via_ctypes

        hook = _ntff_profile_via_ctypes("/opt/axon/libaxon_pjrt.so")
    except Exception:
        hook = None
    mod = types.ModuleType("antenv.axon_hooks")
    mod.get_axon_ntff_profile_hook = lambda: hook
    mod.set_axon_ntff_profile_hook = lambda h: None
    sys.modules["antenv.axon_hooks"] = mod


_install_ntff_shim()

import concourse.bass as bass  # noqa: E402
import concourse.mybir as mybir  # noqa: E402
import concourse.tile as tile  # noqa: E402
from concourse.bass_utils import run_bass_kernel_spmd  # noqa: E402
from concourse.vector_clock import ScopedClock  # noqa: E402


def _patched_drain_and_barrier(self, tick_clock, wait_clock):
    """The walrus build in this image rejects a Drain carrying multiple sem
    waits ("Too many sync wait commands").  Emit the kernel-tail waits as
    standalone wait instructions on the sync engine instead, then drain."""
    nc = self.nc
    probe = nc.sync.nop(nofuse=True)
    wait_clock.add_sem_waits(probe.ins, ScopedClock({None: tick_clock.global_clock}))
    waits = list(probe.ins.sync_info.on_wait)
    probe.ins.sync_info.on_wait.clear()
    name2sem = {s.name: s for s in self.sems.allocated().values()}
    for w in waits:
        nc.sync.wait_ge(name2sem[w.ant_name], w.wait_value)
    nc.sync.drain()
    nc.all_engine_barrier()
    popped = nc._tile_sem_poison_stack.pop()
    assert popped is self._sem_poison
    nc.clear_and_free_semaphores(list(self.sems.allocated().values()))
    nc.all_engine_barrier()


tile.TileContext._drain_and_barrier = _patched_drain_and_barrier

# Max sem-waits this walrus build accepts on a single instruction.
_WAIT_LIMIT = 1


def _split_waits(nc, limit=_WAIT_LIMIT):
    """Hoist excess per-instruction sem waits into standalone EventSemaphore
    instructions (same engine, immediately preceding), since this walrus build
    rejects instructions carrying more than a couple of sync waits."""
    n_split = 0
    for f in nc.m.functions:
        for bb in f.blocks:
            insts = bb.instructions
            i = 0
            while i < len(insts):
                inst = insts[i]
                si = getattr(inst, "sync_info", None)
                waits = si.on_wait if si is not None else None
                lim = 1 if type(inst).__name__ in ("InstDrain", "InstNoOp") else limit
                if waits is not None and len(waits) > lim:
                    excess = list(waits)[lim:]
                    del waits[lim:]
                    for w in excess:
                        ev = mybir.InstEventSemaphore(
                            name=f"I-{nc.next_id()}",
                            engine=inst.engine,
                            ins=[],
                            outs=[],
                        )
                        ev.sync_info = mybir.SyncInfo(on_wait=[w], on_update=[])
                        insts.insert(i, ev)
                        i += 1
                        n_split += 1
                i += 1
    return n_split

# ---------------------------------------------------------------------------
# Problem constants (hardcoded per the harness contract)
# ---------------------------------------------------------------------------
B = 2
SQ = 2048
SKV = 2048
D = 1024
NH = 16
DK = 64

NCORES = 8
GSZ = 4  # cores per batch group
HLOC = 4  # heads per core
DLOC = HLOC * DK  # 256 local context channels
QB = SQ // GSZ  # 512 query rows owned per core (for o-proj/LN)
P = 128
QCH = 512  # q chunk (matmul moving free dim)
NQC = SQ // QCH  # 4
NKT = SKV // P  # 16 k tiles
NMT = D // P  # 8 contraction tiles over model dim

F32 = mybir.dt.float32
BF16 = mybir.dt.bfloat16

LN_EPS = 1e-5

_CACHE = {}
LAST_RESULT = None


def _build():
    """Build the SPMD Bass program (identical on all 8 cores)."""
    nc = bass.Bass("TRN2", target_bir_lowering=False, num_devices=NCORES)

    # ---- kernel I/O (per-core shards supplied by the host) ----
    xqT = nc.dram_tensor("xqT", [D, SQ], BF16, kind="ExternalInput")
    xkvT = nc.dram_tensor("xkvT", [D, SKV], BF16, kind="ExternalInput")
    wqT = nc.dram_tensor("wqT", [D, DLOC], BF16, kind="ExternalInput")
    wkT = nc.dram_tensor("wkT", [D, DLOC], BF16, kind="ExternalInput")
    wvT = nc.dram_tensor("wvT", [D, DLOC], BF16, kind="ExternalInput")
    bqs = nc.dram_tensor("bqs", [P, 2], F32, kind="ExternalInput")
    bks = nc.dram_tensor("bks", [P, 2], F32, kind="ExternalInput")
    bvr = nc.dram_tensor("bvr", [1, DLOC], BF16, kind="ExternalInput")
    woT = nc.dram_tensor("woT", [DLOC, D], BF16, kind="ExternalInput")
    qres = nc.dram_tensor("qres", [QB, D], F32, kind="ExternalInput")
    gam = nc.dram_tensor("gam", [P, D], F32, kind="ExternalInput")
    bet = nc.dram_tensor("bet", [P, D], F32, kind="ExternalInput")
    out = nc.dram_tensor("out", [QB, D], F32, kind="ExternalOutput")

    groups = [[0, 1, 2, 3], [4, 5, 6, 7]]

    with tile.TileContext(nc) as tc:
        with (
            tc.tile_pool(name="cpool", bufs=1) as cpool,
            tc.tile_pool(name="spool", bufs=2) as spool,
            tc.tile_pool(name="dram", bufs=1, space="DRAM") as dram,
        ):
            # ---- persistent SBUF tensors ----
            wq_sb = cpool.tile([P, NMT, DLOC], BF16)
            wk_sb = cpool.tile([P, NMT, DLOC], BF16)
            wv_sb = cpool.tile([P, NMT, DLOC], BF16)
            bqs_sb = cpool.tile([P, 2], F32)
            bks_sb = cpool.tile([P, 2], F32)
            bvr_sb = cpool.tile([1, DLOC], BF16)
            onesP = cpool.tile([P, P], BF16)
            qt_sb = cpool.tile([P, 2, SQ], BF16)  # Q'^T  (d on partitions)
            kt_sb = cpool.tile([P, 2, SKV], BF16)  # K'^T
            v_sb = cpool.tile([P, NKT, DLOC], BF16)  # V rows (k on partitions)
            # normalized local context C^T: [d(128) x head-pair x q]
            ct_sb = cpool.tile([P, 2, SQ], BF16)

            nc.sync.dma_start(wq_sb[:], wqT.ap().rearrange("(t p) d -> p t d", p=P))
            nc.sync.dma_start(wk_sb[:], wkT.ap().rearrange("(t p) d -> p t d", p=P))
            nc.sync.dma_start(wv_sb[:], wvT.ap().rearrange("(t p) d -> p t d", p=P))
            nc.sync.dma_start(bqs_sb[:], bqs.ap())
            nc.sync.dma_start(bks_sb[:], bks.ap())
            nc.sync.dma_start(bvr_sb[:], bvr.ap())
            eps_sb = cpool.tile([P, 1], F32)
            nc.vector.memset(onesP[:], 1.0)
            nc.vector.memset(eps_sb[:], LN_EPS)

            # ---------------- Phase A: projections ----------------
            with (
                tc.tile_pool(name="xpool", bufs=1) as xpool,
                tc.tile_pool(name="psA", bufs=8, space="PSUM") as psA,
            ):
                xq_sb = xpool.tile([P, NMT, SQ], BF16)
                xkv_sb = xpool.tile([P, NMT, SKV], BF16)
                nc.sync.dma_start(
                    xq_sb[:], xqT.ap().rearrange("(t p) q -> p t q", p=P)
                )
                nc.sync.dma_start(
                    xkv_sb[:], xkvT.ap().rearrange("(t p) q -> p t q", p=P)
                )

                # Q'^T and K'^T: out[d_tile(128), q(512)] accumulated over 8 m-tiles.
                for (x_sb, w_sb, b_sb, dst, scale) in (
                    (xq_sb, wq_sb, bqs_sb, qt_sb, 0.125),
                    (xkv_sb, wk_sb, bks_sb, kt_sb, 1.0),
                ):
                    for dt in range(2):
                        for qc in range(NQC):
                            ps = psA.tile([P, QCH], F32, tag="pj")
                            for mt in range(NMT):
                                nc.tensor.matmul(
                                    ps[:],
                                    lhsT=w_sb[:, mt, P * dt : P * dt + P],
                                    rhs=x_sb[:, mt, QCH * qc : QCH * qc + QCH],
                                    start=(mt == 0),
                                    stop=(mt == NMT - 1),
                                )
                            nc.scalar.activation(
                                dst[:, dt, QCH * qc : QCH * qc + QCH],
                                ps[:],
                                mybir.ActivationFunctionType.Identity,
                                bias=b_sb[:, dt : dt + 1],
                                scale=scale,
                            )

                # V: out[k_tile(128), d_loc(256)] accumulated over m-tiles + bias.
                for kt in range(NKT):
                    ps = psA.tile([P, QCH], F32, tag="pj")
                    pv = ps[:, :DLOC]
                    for mt in range(NMT):
                        nc.tensor.matmul(
                            pv,
                            lhsT=xkv_sb[:, mt, P * kt : P * kt + P],
                            rhs=wv_sb[:, mt, :],
                            start=(mt == 0),
                            stop=False,
                        )
                    nc.tensor.matmul(
                        pv,
                        lhsT=onesP[0:1, :],
                        rhs=bvr_sb[0:1, :],
                        start=False,
                        stop=True,
                    )
                    nc.vector.tensor_copy(v_sb[:, kt, :], pv)

            # ---------------- Phase B: attention ----------------
            with (
                tc.tile_pool(name="bpool", bufs=1) as bpool,
                tc.tile_pool(name="psB", bufs=1, space="PSUM") as psB,
            ):
                for jc in range(NQC):
                    for hp in range(2):
                        h0, h1 = 2 * hp, 2 * hp + 1
                        p0 = bpool.tile([P, NKT, QCH], BF16, tag="p0")
                        p1 = bpool.tile([P, NKT, QCH], BF16, tag="p1")
                        # context: h0 in partitions 0:64, h1 in 64:128
                        ctx = psB.tile([P, QCH], F32, tag="ctx", bufs=2)
                        # denominators: h0 in row 0, h1 in row 64
                        dn = psB.tile([P, QCH], F32, tag="dn", bufs=1)
                        qsl = slice(QCH * jc, QCH * jc + QCH)

                        def scores(kt):
                            s0 = psB.tile([P, QCH], F32, tag="s0", bufs=2)
                            s1 = psB.tile([P, QCH], F32, tag="s1", bufs=2)
                            ksl = slice(P * kt, P * kt + P)
                            nc.tensor.matmul(
                                s0[:], lhsT=kt_sb[0:DK, hp, ksl],
                                rhs=qt_sb[0:DK, hp, qsl],
                            )
                            nc.tensor.matmul(
                                s1[:], lhsT=kt_sb[DK:P, hp, ksl],
                                rhs=qt_sb[DK:P, hp, qsl],
                            )
                            nc.scalar.activation(
                                p0[:, kt, :], s0[:], mybir.ActivationFunctionType.Exp
                            )
                            nc.scalar.activation(
                                p1[:, kt, :], s1[:], mybir.ActivationFunctionType.Exp
                            )

                        def ctxmm(kt):
                            st, sp = kt == 0, kt == NKT - 1
                            nc.tensor.matmul(
                                ctx[0:DK, :],
                                lhsT=v_sb[:, kt, DK * h0 : DK * h0 + DK],
                                rhs=p0[:, kt, :],
                                start=st,
                                stop=sp,
                            )
                            nc.tensor.matmul(
                                ctx[DK:P, :],
                                lhsT=v_sb[:, kt, DK * h1 : DK * h1 + DK],
                                rhs=p1[:, kt, :],
                                start=st,
                                stop=sp,
                            )
                            nc.tensor.matmul(
                                dn[0:1, :],
                                lhsT=onesP[:, 0:1],
                                rhs=p0[:, kt, :],
                                start=st,
                                stop=sp,
                            )
                            nc.tensor.matmul(
                                dn[DK : DK + 1, :],
                                lhsT=onesP[:, 0:1],
                                rhs=p1[:, kt, :],
                                start=st,
                                stop=sp,
                            )

                        # software-pipeline: scores(kt+1) before ctx(kt)
                        scores(0)
                        for kt in range(1, NKT):
                            scores(kt)
                            ctxmm(kt - 1)
                        ctxmm(NKT - 1)

                        # normalize each head's context by its softmax denom:
                        # reciprocal -> bf16 -> ones outer-product broadcast
                        # (PE) -> copy to SBUF (ACT) -> multiply (DVE)
                        rd = spool.tile([P, QCH], F32, tag="rd")
                        rdb = spool.tile([P, QCH], BF16, tag="rdb")
                        bcp = psB.tile([P, QCH], F32, tag="bc", bufs=1)
                        bc_sb = spool.tile([P, QCH], F32, tag="bcs")
                        nc.vector.reciprocal(rd[0:1, :], dn[0:1, :])
                        nc.vector.reciprocal(rd[DK : DK + 1, :], dn[DK : DK + 1, :])
                        nc.vector.tensor_copy(rdb[0:1, :], rd[0:1, :])
                        nc.vector.tensor_copy(rdb[DK : DK + 1, :], rd[DK : DK + 1, :])
                        nc.tensor.matmul(
                            bcp[0:DK, :], lhsT=onesP[0:1, 0:DK], rhs=rdb[0:1, :]
                        )
                        nc.tensor.matmul(
                            bcp[DK:P, :],
                            lhsT=onesP[DK : DK + 1, 0:DK],
                            rhs=rdb[DK : DK + 1, :],
                        )
                        nc.scalar.activation(
                            bc_sb[:], bcp[:], mybir.ActivationFunctionType.Copy
                        )
                        nc.vector.tensor_mul(
                            ct_sb[0:DK, hp, qsl], ctx[0:DK, :], bc_sb[0:DK, :]
                        )
                        nc.vector.tensor_mul(
                            ct_sb[DK:P, hp, qsl], ctx[DK:P, :], bc_sb[DK:P, :]
                        )

            # ------- Phase C: partial o-proj + ReduceScatter + residual/LN -------
            with (
                tc.tile_pool(name="opool", bufs=1) as opool,
                tc.tile_pool(name="psC", bufs=4, space="PSUM") as psC,
            ):
                cc_in = dram.tile([SQ, D], BF16)
                cc_rs = dram.tile([QB, D], BF16)

                wo_sb = opool.tile([P, 2, D], BF16)
                qres_sb = opool.tile([P, GSZ, D], F32)
                gam_sb = opool.tile([P, D], F32)
                bet_sb = opool.tile([P, D], F32)
                nc.sync.dma_start(wo_sb[:], woT.ap().rearrange("(t p) n -> p t n", p=P))
                nc.sync.dma_start(
                    qres_sb[:], qres.ap().rearrange("(t p) n -> p t n", p=P)
                )
                nc.sync.dma_start(gam_sb[:], gam.ap())
                nc.sync.dma_start(bet_sb[:], bet.ap())

                # partial o-proj over ALL 2048 rows (local 256 channels)
                cc_in_v = cc_in.rearrange("(t p) n -> p t n", p=P)
                for qt in range(SQ // P):
                    for nch in range(2):
                        po = psC.tile([P, QCH], F32, tag="po")
                        nsl = slice(QCH * nch, QCH * nch + QCH)
                        for dt2 in range(2):
                            nc.tensor.matmul(
                                po[:],
                                lhsT=ct_sb[:, dt2, P * qt : P * qt + P],
                                rhs=wo_sb[:, dt2, nsl],
                                start=(dt2 == 0),
                                stop=(dt2 == 1),
                            )
                        op_sb = opool.tile([P, QCH], BF16, tag="op", bufs=3)
                        nc.vector.tensor_copy(op_sb[:], po[:])
                        nc.sync.dma_start(cc_in_v[:, qt, nsl], op_sb[:])

                nc.gpsimd.collective_compute(
                    "ReduceScatter",
                    mybir.AluOpType.add,
                    replica_groups=groups,
                    ins=[cc_in.opt()],
                    outs=[cc_rs.opt()],
                )

                rs_sb = opool.tile([P, GSZ, D], BF16)
                nc.sync.dma_start(
                    rs_sb[:], cc_rs.rearrange("(t p) n -> p t n", p=P)
                )

                for qt in range(GSZ):
                    x_sb = opool.tile([P, D], F32, tag="x", bufs=2)
                    nc.vector.tensor_add(x_sb[:], rs_sb[:, qt, :], qres_sb[:, qt, :])

                    # LayerNorm over the full 1024-wide row
                    mean = spool.tile([P, 1], F32, tag="mean")
                    nmean = spool.tile([P, 1], F32, tag="nmean")
                    xc = opool.tile([P, D], F32, tag="xc", bufs=2)
                    sq = opool.tile([P, D], F32, tag="sq", bufs=2)
                    ssq = spool.tile([P, 1], F32, tag="ssq")
                    sd = spool.tile([P, 1], F32, tag="sd")
                    rstd = spool.tile([P, 1], F32, tag="rstd")
                    y_sb = opool.tile([P, D], F32, tag="y", bufs=2)

                    nc.vector.reduce_sum(mean[:], x_sb[:], axis=mybir.AxisListType.X)
                    nc.scalar.mul(nmean[:], mean[:], -1.0 / D)
                    nc.vector.tensor_scalar_add(xc[:], x_sb[:], nmean[:])
                    nc.scalar.activation(
                        sq[:],
                        xc[:],
                        mybir.ActivationFunctionType.Square,
                        accum_out=ssq[:],
                    )
                    nc.scalar.activation(
                        sd[:],
                        ssq[:],
                        mybir.ActivationFunctionType.Sqrt,
                        scale=1.0 / D,
                        bias=eps_sb[:],
                    )
                    nc.vector.reciprocal(rstd[:], sd[:])
                    nc.vector.tensor_scalar_mul(y_sb[:], xc[:], rstd[:])
                    nc.vector.tensor_mul(y_sb[:], y_sb[:], gam_sb[:])
                    nc.vector.tensor_add(y_sb[:], y_sb[:], bet_sb[:])
                    nc.sync.dma_start(
                        out.ap().rearrange("(t p) n -> p t n", p=P)[:, qt, :], y_sb[:]
                    )

    _split_waits(nc)
    return nc


def _prep_inputs(query, key_value, W_qkv, b_qkv, W_o, b_o, ln_gamma, ln_beta):
    bf16 = ml_dtypes.bfloat16
    f32 = np.float32
    query = np.asarray(query, f32)
    key_value = np.asarray(key_value, f32)
    W_qkv = np.asarray(W_qkv, f32)
    b_qkv = np.asarray(b_qkv, f32)
    W_o = np.asarray(W_o, f32)
    b_o = np.asarray(b_o, f32)
    ln_gamma = np.asarray(ln_gamma, f32)
    ln_beta = np.asarray(ln_beta, f32)

    Wq, Wk, Wv = W_qkv[:D], W_qkv[D : 2 * D], W_qkv[2 * D :]
    bq, bk, bv = b_qkv[:D], b_qkv[D : 2 * D], b_qkv[2 * D :]

    woT_full = np.ascontiguousarray(W_o.T)  # [d_in, n_out]
    gam = np.ascontiguousarray(np.broadcast_to(ln_gamma, (P, D))).astype(f32)
    bet = np.ascontiguousarray(np.broadcast_to(ln_beta, (P, D))).astype(f32)

    xqT = [np.ascontiguousarray(query[b].T).astype(bf16) for b in range(B)]
    xkvT = [np.ascontiguousarray(key_value[b].T).astype(bf16) for b in range(B)]

    in_maps = []
    for c in range(NCORES):
        b = c // GSZ
        hb = c % GSZ
        jb = c % GSZ
        sl = slice(DLOC * hb, DLOC * hb + DLOC)
        in_maps.append(
            {
                "xqT": xqT[b],
                "xkvT": xkvT[b],
                "wqT": np.ascontiguousarray(Wq[sl].T).astype(bf16),
                "wkT": np.ascontiguousarray(Wk[sl].T).astype(bf16),
                "wvT": np.ascontiguousarray(Wv[sl].T).astype(bf16),
                "bqs": np.ascontiguousarray(
                    (bq[sl] * 0.125).reshape(2, P).T
                ).astype(f32),
                "bks": np.ascontiguousarray(bk[sl].reshape(2, P).T).astype(f32),
                "bvr": bv[sl][None, :].astype(bf16),
                "woT": np.ascontiguousarray(woT_full[sl]).astype(bf16),
                "qres": (query[b, QB * jb : QB * jb + QB] + b_o[None, :]).astype(f32),
                "gam": gam,
                "bet": bet,
            }
        )
    return in_maps


def kernel(query, key_value, W_qkv, b_qkv, W_o, b_o, ln_gamma, ln_beta):
    global LAST_RESULT
    if "nc" not in _CACHE:
        _CACHE["nc"] = _build()
    nc = _CACHE["nc"]
    in_maps = _prep_inputs(
        query, key_value, W_qkv, b_qkv, W_o, b_o, ln_gamma, ln_beta
    )
    res = run_bass_kernel_spmd(nc, in_maps, core_ids=list(range(NCORES)))
    LAST_RESULT = res
    full = np.empty((B, SQ, D), np.float32)
    for c in range(NCORES):
        b = c // GSZ
        jb = c % GSZ
        full[b, QB * jb : QB * jb + QB] = res.results[c]["out"]
    return full


# revision 24
# speedup vs baseline: 1.2250x; 1.2250x over previous
"""Memory-efficient multi-head cross-attention on 8 TRN2 NeuronCores.

Sharding: batch (2) x head-block (4 heads each) across 8 cores, tensor-parallel
qkv projections; each core computes a partial o-projection over all 2048 query
rows from its 4 heads' context, and a per-q-chunk ReduceScatter inside each
4-core batch group sums the partials and hands every core 128 rows per chunk,
on which residual + LayerNorm run locally.

kernel(**inputs) takes the FULL unsharded inputs and returns the FULL output.
"""

import sys
import types

import ml_dtypes
import numpy as np

# ---------------------------------------------------------------------------
# Environment shims (must run before concourse imports are used)
# ---------------------------------------------------------------------------


def _install_ntff_shim():
    """Provide antenv.axon_hooks (absent in this image) so that
    run_bass_kernel_spmd(trace=True) can capture NTFF profiles via the
    axon ctypes hook. Harmless when tracing is off."""
    if "antenv.axon_hooks" in sys.modules:
        return
    hook = None
    try:
        from trn_agent_boot.trn_boot import _ntff_profile_via_ctypes

        hook = _ntff_profile_via_ctypes("/opt/axon/libaxon_pjrt.so")
    except Exception:
        hook = None
    mod = types.ModuleType("antenv.axon_hooks")
    mod.get_axon_ntff_profile_hook = lambda: hook
    mod.set_axon_ntff_profile_hook = lambda h: None
    sys.modules["antenv.axon_hooks"] = mod


_install_ntff_shim()

import concourse.bass as bass  # noqa: E402
import concourse.mybir as mybir  # noqa: E402
import concourse.tile as tile  # noqa: E402
from concourse.bass_utils import run_bass_kernel_spmd  # noqa: E402
from concourse.vector_clock import ScopedClock  # noqa: E402


def _patched_drain_and_barrier(self, tick_clock, wait_clock):
    """The walrus build in this image rejects a Drain carrying multiple sem
    waits ("Too many sync wait commands").  Emit the kernel-tail waits as
    standalone wait instructions on the sync engine instead, then drain."""
    nc = self.nc
    probe = nc.sync.nop(nofuse=True)
    wait_clock.add_sem_waits(probe.ins, ScopedClock({None: tick_clock.global_clock}))
    waits = list(probe.ins.sync_info.on_wait)
    probe.ins.sync_info.on_wait.clear()
    name2sem = {s.name: s for s in self.sems.allocated().values()}
    for w in waits:
        nc.sync.wait_ge(name2sem[w.ant_name], w.wait_value)
    nc.sync.drain()
    nc.all_engine_barrier()
    popped = nc._tile_sem_poison_stack.pop()
    assert popped is self._sem_poison
    nc.clear_and_free_semaphores(list(self.sems.allocated().values()))
    nc.all_engine_barrier()


tile.TileContext._drain_and_barrier = _patched_drain_and_barrier

# Max sem-waits this walrus build accepts on a single instruction.
_WAIT_LIMIT = 1


def _split_waits(nc, limit=_WAIT_LIMIT):
    """Hoist excess per-instruction sem waits into standalone EventSemaphore
    instructions (same engine, immediately preceding), since this walrus build
    rejects instructions carrying more than one sync wait."""
    n_split = 0
    for f in nc.m.functions:
        for bb in f.blocks:
            insts = bb.instructions
            i = 0
            while i < len(insts):
                inst = insts[i]
                si = getattr(inst, "sync_info", None)
                waits = si.on_wait if si is not None else None
                if waits is not None and len(waits) > limit:
                    excess = list(waits)[limit:]
                    del waits[limit:]
                    for w in excess:
                        ev = mybir.InstEventSemaphore(
                            name=f"I-{nc.next_id()}",
                            engine=inst.engine,
                            ins=[],
                            outs=[],
                        )
                        ev.sync_info = mybir.SyncInfo(on_wait=[w], on_update=[])
                        insts.insert(i, ev)
                        i += 1
                        n_split += 1
                i += 1
    return n_split


# ---------------------------------------------------------------------------
# Problem constants (hardcoded per the harness contract)
# ---------------------------------------------------------------------------
B = 2
SQ = 2048
SKV = 2048
D = 1024
NH = 16
DK = 64

NCORES = 8
GSZ = 4  # cores per batch group
HLOC = 4  # heads per core
DLOC = HLOC * DK  # 256 local context channels
P = 128
QCH = 512  # q chunk (matmul moving free dim)
NQC = SQ // QCH  # 4
NKT = SKV // P  # 16 k tiles
NMT = D // P  # 8 contraction tiles over model dim

F32 = mybir.dt.float32
BF16 = mybir.dt.bfloat16

LN_EPS = 1e-5

_CACHE = {}
LAST_RESULT = None


def _build():
    """Build the SPMD Bass program (identical on all 8 cores)."""
    nc = bass.Bass("TRN2", target_bir_lowering=False, num_devices=NCORES)

    # ---- kernel I/O (per-core shards supplied by the host) ----
    xqT = nc.dram_tensor("xqT", [D, SQ], BF16, kind="ExternalInput")
    xkvT = nc.dram_tensor("xkvT", [D, SKV], BF16, kind="ExternalInput")
    wqT = nc.dram_tensor("wqT", [D, DLOC], BF16, kind="ExternalInput")
    wkT = nc.dram_tensor("wkT", [D, DLOC], BF16, kind="ExternalInput")
    wvT = nc.dram_tensor("wvT", [D, DLOC], BF16, kind="ExternalInput")
    bqs = nc.dram_tensor("bqs", [P, 2], F32, kind="ExternalInput")
    bks = nc.dram_tensor("bks", [P, 2], F32, kind="ExternalInput")
    bvr = nc.dram_tensor("bvr", [1, DLOC], BF16, kind="ExternalInput")
    woT = nc.dram_tensor("woT", [DLOC, D], BF16, kind="ExternalInput")
    # residual rows (query + b_o) for this core: [jc, 128, D]
    qres = nc.dram_tensor("qres", [NQC, P, D], F32, kind="ExternalInput")
    gam = nc.dram_tensor("gam", [P, D], F32, kind="ExternalInput")
    bet = nc.dram_tensor("bet", [P, D], F32, kind="ExternalInput")
    out = nc.dram_tensor("out", [NQC, P, D], F32, kind="ExternalOutput")

    groups = [[0, 1, 2, 3], [4, 5, 6, 7]]
    Exp = mybir.ActivationFunctionType.Exp

    with tile.TileContext(nc) as tc:
        with (
            tc.tile_pool(name="cpool", bufs=1) as cpool,
            tc.tile_pool(name="spool", bufs=2) as spool,
            tc.tile_pool(name="dram", bufs=1, space="DRAM") as dram,
        ):
            # ---- persistent SBUF tensors ----
            wq_sb = cpool.tile([P, NMT, DLOC], BF16)
            wk_sb = cpool.tile([P, NMT, DLOC], BF16)
            wv_sb = cpool.tile([P, NMT, DLOC], BF16)
            bqs_sb = cpool.tile([P, 2], F32)
            bks_sb = cpool.tile([P, 2], F32)
            bvr_sb = cpool.tile([1, DLOC], BF16)
            onesP = cpool.tile([P, P], BF16)
            eps_sb = cpool.tile([P, 1], F32)
            qt_sb = cpool.tile([P, 2, SQ], BF16)  # Q'^T  (d on partitions)
            kt_sb = cpool.tile([P, 2, SKV], BF16)  # K'^T
            v_sb = cpool.tile([P, NKT, DLOC], BF16)  # V rows (k on partitions)
            # normalized local context C^T: [d(128) x head-pair x q]
            ct_sb = cpool.tile([P, 2, SQ], BF16)

            nc.sync.dma_start(wk_sb[:], wkT.ap().rearrange("(t p) d -> p t d", p=P))
            nc.sync.dma_start(wq_sb[:], wqT.ap().rearrange("(t p) d -> p t d", p=P))
            nc.sync.dma_start(wv_sb[:], wvT.ap().rearrange("(t p) d -> p t d", p=P))
            nc.sync.dma_start(bqs_sb[:], bqs.ap())
            nc.sync.dma_start(bks_sb[:], bks.ap())
            nc.sync.dma_start(bvr_sb[:], bvr.ap())
            nc.vector.memset(onesP[:], 1.0)
            nc.vector.memset(eps_sb[:], LN_EPS)

            # ---------------- Phase A: projections ----------------
            with (
                tc.tile_pool(name="xpool", bufs=1) as xpool,
                tc.tile_pool(name="psA", bufs=8, space="PSUM") as psA,
            ):
                xkv_sb = xpool.tile([P, NMT, SKV], BF16)
                xq_sb = xpool.tile([P, NMT, SQ], BF16)
                nc.sync.dma_start(
                    xkv_sb[:], xkvT.ap().rearrange("(t p) q -> p t q", p=P)
                )
                nc.sync.dma_start(
                    xq_sb[:], xqT.ap().rearrange("(t p) q -> p t q", p=P)
                )

                # K'^T then Q'^T: out[d_tile(128), q(512)] over 8 m-tiles;
                # copyback on DVE: (psum * scale) + bias -> bf16
                for (x_sb, w_sb, b_sb, dst, scale) in (
                    (xkv_sb, wk_sb, bks_sb, kt_sb, 1.0),
                    (xq_sb, wq_sb, bqs_sb, qt_sb, 0.125),
                ):
                    for dt in range(2):
                        for qc in range(NQC):
                            ps = psA.tile([P, QCH], F32, tag="pj")
                            for mt in range(NMT):
                                nc.tensor.matmul(
                                    ps[:],
                                    lhsT=w_sb[:, mt, P * dt : P * dt + P],
                                    rhs=x_sb[:, mt, QCH * qc : QCH * qc + QCH],
                                    start=(mt == 0),
                                    stop=(mt == NMT - 1),
                                )
                            nc.vector.tensor_scalar(
                                dst[:, dt, QCH * qc : QCH * qc + QCH],
                                ps[:],
                                scale,
                                b_sb[:, dt : dt + 1],
                                mybir.AluOpType.mult,
                                mybir.AluOpType.add,
                            )

                # V: out[k_tile(128), d_loc(256)] over m-tiles + bias row.
                for kt in range(NKT):
                    ps = psA.tile([P, QCH], F32, tag="pj")
                    pv = ps[:, :DLOC]
                    for mt in range(NMT):
                        nc.tensor.matmul(
                            pv,
                            lhsT=xkv_sb[:, mt, P * kt : P * kt + P],
                            rhs=wv_sb[:, mt, :],
                            start=(mt == 0),
                            stop=False,
                        )
                    nc.tensor.matmul(
                        pv,
                        lhsT=onesP[0:1, :],
                        rhs=bvr_sb[0:1, :],
                        start=False,
                        stop=True,
                    )
                    nc.vector.tensor_copy(v_sb[:, kt, :], pv)

            # ------- Phase B+C: attention, o-proj, chunked RS, LN -------
            with (
                tc.tile_pool(name="bpool", bufs=1) as bpool,
                tc.tile_pool(name="opool", bufs=1) as opool,
                tc.tile_pool(name="psB", bufs=1, space="PSUM") as psB,
            ):
                wo_sb = opool.tile([P, 2, D], BF16)
                qres_sb = opool.tile([P, NQC, D], F32)
                gam_sb = opool.tile([P, D], F32)
                bet_sb = opool.tile([P, D], F32)
                nc.sync.dma_start(wo_sb[:], woT.ap().rearrange("(t p) n -> p t n", p=P))
                nc.sync.dma_start(qres_sb[:], qres.ap().rearrange("j p n -> p j n"))
                nc.sync.dma_start(gam_sb[:], gam.ap())
                nc.sync.dma_start(bet_sb[:], bet.ap())

                for jc in range(NQC):
                    qsl = slice(QCH * jc, QCH * jc + QCH)
                    for hp in range(2):
                        h0, h1 = 2 * hp, 2 * hp + 1
                        p0 = bpool.tile([P, NKT, QCH], BF16, tag="p0")
                        p1 = bpool.tile([P, NKT, QCH], BF16, tag="p1")
                        # context: h0 in partitions 0:64, h1 in 64:128
                        ctx = psB.tile([P, QCH], F32, tag="ctx", bufs=1)
                        # denominators: h0 in row 0, h1 in row 64
                        dn = psB.tile([P, QCH], F32, tag="dn", bufs=2)

                        def scores2(ktp):
                            # two k-tiles per psum slab -> one wide Exp each
                            s0 = psB.tile([P, 2, QCH], F32, tag="s0", bufs=1)
                            s1 = psB.tile([P, 2, QCH], F32, tag="s1", bufs=1)
                            for j in range(2):
                                kt = 2 * ktp + j
                                ksl = slice(P * kt, P * kt + P)
                                nc.tensor.matmul(
                                    s0[:, j, :],
                                    lhsT=kt_sb[0:DK, hp, ksl],
                                    rhs=qt_sb[0:DK, hp, qsl],
                                )
                                nc.tensor.matmul(
                                    s1[:, j, :],
                                    lhsT=kt_sb[DK:P, hp, ksl],
                                    rhs=qt_sb[DK:P, hp, qsl],
                                )
                            ksl2 = slice(2 * ktp, 2 * ktp + 2)
                            nc.scalar.activation(p0[:, ksl2, :], s0[:], Exp)
                            nc.scalar.activation(p1[:, ksl2, :], s1[:], Exp)

                        def ctxmm(kt):
                            st, sp = kt == 0, kt == NKT - 1
                            nc.tensor.matmul(
                                ctx[0:DK, :],
                                lhsT=v_sb[:, kt, DK * h0 : DK * h0 + DK],
                                rhs=p0[:, kt, :],
                                start=st,
                                stop=sp,
                            )
                            nc.tensor.matmul(
                                ctx[DK:P, :],
                                lhsT=v_sb[:, kt, DK * h1 : DK * h1 + DK],
                                rhs=p1[:, kt, :],
                                start=st,
                                stop=sp,
                            )
                            nc.tensor.matmul(
                                dn[0:1, :],
                                lhsT=onesP[:, 0:1],
                                rhs=p0[:, kt, :],
                                start=st,
                                stop=sp,
                            )
                            nc.tensor.matmul(
                                dn[DK : DK + 1, :],
                                lhsT=onesP[:, 0:1],
                                rhs=p1[:, kt, :],
                                start=st,
                                stop=sp,
                            )

                        # software pipeline: scores(ktp) ahead of ctx
                        scores2(0)
                        for ktp in range(1, NKT // 2):
                            scores2(ktp)
                            ctxmm(2 * ktp - 2)
                            ctxmm(2 * ktp - 1)
                        ctxmm(NKT - 2)
                        ctxmm(NKT - 1)

                        # normalize each head's context by its softmax denom:
                        # fast reciprocal -> bf16 -> ones outer-product (PE)
                        # -> copy to SBUF -> multiply (DVE)
                        rd = spool.tile([P, QCH], F32, tag="rd")
                        rdb = spool.tile([P, QCH], BF16, tag="rdb")
                        bcp = psB.tile([P, QCH], F32, tag="po", bufs=1)
                        bc_sb = spool.tile([P, QCH], F32, tag="bcs")
                        nc.vector.reciprocal(rd[0:1, :], dn[0:1, :])
                        nc.vector.reciprocal(rd[DK : DK + 1, :], dn[DK : DK + 1, :])
                        nc.vector.tensor_copy(rdb[0:1, :], rd[0:1, :])
                        nc.vector.tensor_copy(rdb[DK : DK + 1, :], rd[DK : DK + 1, :])
                        nc.tensor.matmul(
                            bcp[0:DK, :], lhsT=onesP[0:1, 0:DK], rhs=rdb[0:1, :]
                        )
                        nc.tensor.matmul(
                            bcp[DK:P, :],
                            lhsT=onesP[DK : DK + 1, 0:DK],
                            rhs=rdb[DK : DK + 1, :],
                        )
                        nc.vector.tensor_copy(bc_sb[:], bcp[:])
                        nc.vector.tensor_mul(
                            ct_sb[0:DK, hp, qsl], ctx[0:DK, :], bc_sb[0:DK, :]
                        )
                        nc.vector.tensor_mul(
                            ct_sb[DK:P, hp, qsl], ctx[DK:P, :], bc_sb[DK:P, :]
                        )

                    # ---- partial o-proj for this q chunk + RS + LN ----
                    cc_in = dram.tile([QCH, D], BF16, name=f"cc_in_{jc}")
                    cc_rs = dram.tile([P, D], BF16, name=f"cc_rs_{jc}")
                    cc_in_v = cc_in.rearrange("(t p) n -> p t n", p=P)
                    for qt in range(QCH // P):
                        for nch in range(2):
                            po = psB.tile([P, QCH], F32, tag="po", bufs=1)
                            nsl = slice(QCH * nch, QCH * nch + QCH)
                            qoff = QCH * jc + P * qt
                            for dt2 in range(2):
                                nc.tensor.matmul(
                                    po[:],
                                    lhsT=ct_sb[:, dt2, qoff : qoff + P],
                                    rhs=wo_sb[:, dt2, nsl],
                                    start=(dt2 == 0),
                                    stop=(dt2 == 1),
                                )
                            op_sb = opool.tile([P, QCH], BF16, tag="op", bufs=3)
                            nc.vector.tensor_copy(op_sb[:], po[:])
                            nc.sync.dma_start(cc_in_v[:, qt, nsl], op_sb[:])

                    nc.gpsimd.collective_compute(
                        "ReduceScatter",
                        mybir.AluOpType.add,
                        replica_groups=groups,
                        ins=[cc_in.opt()],
                        outs=[cc_rs.opt()],
                    )

                    # epilogue on our 128 rows of this chunk
                    rs_sb = opool.tile([P, D], BF16, tag="rs", bufs=2)
                    nc.sync.dma_start(rs_sb[:], cc_rs[:])
                    x_sb = opool.tile([P, D], F32, tag="x", bufs=2)
                    nc.vector.tensor_add(x_sb[:], rs_sb[:], qres_sb[:, jc, :])

                    mean = spool.tile([P, 1], F32, tag="mean")
                    nmean = spool.tile([P, 1], F32, tag="nmean")
                    xc = opool.tile([P, D], F32, tag="xc", bufs=2)
                    sq = opool.tile([P, D], F32, tag="sq", bufs=2)
                    ssq = spool.tile([P, 1], F32, tag="ssq")
                    sd = spool.tile([P, 1], F32, tag="sd")
                    rstd = spool.tile([P, 1], F32, tag="rstd")
                    y_sb = opool.tile([P, D], F32, tag="y", bufs=2)

                    nc.vector.reduce_sum(mean[:], x_sb[:], axis=mybir.AxisListType.X)
                    nc.scalar.mul(nmean[:], mean[:], -1.0 / D)
                    nc.vector.tensor_scalar_add(xc[:], x_sb[:], nmean[:])
                    nc.scalar.activation(
                        sq[:],
                        xc[:],
                        mybir.ActivationFunctionType.Square,
                        accum_out=ssq[:],
                    )
                    nc.scalar.activation(
                        sd[:],
                        ssq[:],
                        mybir.ActivationFunctionType.Sqrt,
                        scale=1.0 / D,
                        bias=eps_sb[:],
                    )
                    nc.vector.reciprocal(rstd[:], sd[:])
                    nc.vector.tensor_scalar_mul(y_sb[:], xc[:], rstd[:])
                    nc.vector.tensor_mul(y_sb[:], y_sb[:], gam_sb[:])
                    nc.vector.tensor_add(y_sb[:], y_sb[:], bet_sb[:])
                    nc.sync.dma_start(out.ap()[jc], y_sb[:])

    _split_waits(nc)
    return nc


def _prep_inputs(query, key_value, W_qkv, b_qkv, W_o, b_o, ln_gamma, ln_beta):
    bf16 = ml_dtypes.bfloat16
    f32 = np.float32
    query = np.asarray(query, f32)
    key_value = np.asarray(key_value, f32)
    W_qkv = np.asarray(W_qkv, f32)
    b_qkv = np.asarray(b_qkv, f32)
    W_o = np.asarray(W_o, f32)
    b_o = np.asarray(b_o, f32)
    ln_gamma = np.asarray(ln_gamma, f32)
    ln_beta = np.asarray(ln_beta, f32)

    Wq, Wk, Wv = W_qkv[:D], W_qkv[D : 2 * D], W_qkv[2 * D :]
    bq, bk, bv = b_qkv[:D], b_qkv[D : 2 * D], b_qkv[2 * D :]

    woT_full = np.ascontiguousarray(W_o.T)  # [d_in, n_out]
    gam = np.ascontiguousarray(np.broadcast_to(ln_gamma, (P, D))).astype(f32)
    bet = np.ascontiguousarray(np.broadcast_to(ln_beta, (P, D))).astype(f32)

    xqT = [np.ascontiguousarray(query[b].T).astype(bf16) for b in range(B)]
    xkvT = [np.ascontiguousarray(key_value[b].T).astype(bf16) for b in range(B)]

    in_maps = []
    for c in range(NCORES):
        b = c // GSZ
        hb = c % GSZ
        jb = c % GSZ
        sl = slice(DLOC * hb, DLOC * hb + DLOC)
        # this core owns rows 512*jc + 128*jb .. +128 for each chunk jc
        res_rows = np.stack(
            [
                query[b, QCH * jc + P * jb : QCH * jc + P * jb + P] + b_o[None, :]
                for jc in range(NQC)
            ]
        )
        in_maps.append(
            {
                "xqT": xqT[b],
                "xkvT": xkvT[b],
                "wqT": np.ascontiguousarray(Wq[sl].T).astype(bf16),
                "wkT": np.ascontiguousarray(Wk[sl].T).astype(bf16),
                "wvT": np.ascontiguousarray(Wv[sl].T).astype(bf16),
                "bqs": np.ascontiguousarray(
                    (bq[sl] * 0.125).reshape(2, P).T
                ).astype(f32),
                "bks": np.ascontiguousarray(bk[sl].reshape(2, P).T).astype(f32),
                "bvr": bv[sl][None, :].astype(bf16),
                "woT": np.ascontiguousarray(woT_full[sl]).astype(bf16),
                "qres": res_rows.astype(f32),
                "gam": gam,
                "bet": bet,
            }
        )
    return in_maps


def kernel(query, key_value, W_qkv, b_qkv, W_o, b_o, ln_gamma, ln_beta):
    global LAST_RESULT
    if "nc" not in _CACHE:
        _CACHE["nc"] = _build()
    nc = _CACHE["nc"]
    in_maps = _prep_inputs(
        query, key_value, W_qkv, b_qkv, W_o, b_o, ln_gamma, ln_beta
    )
    res = run_bass_kernel_spmd(nc, in_maps, core_ids=list(range(NCORES)))
    LAST_RESULT = res
    full = np.empty((B, SQ, D), np.float32)
    for c in range(NCORES):
        b = c // GSZ
        jb = c % GSZ
        o = res.results[c]["out"]  # [NQC, P, D]
        for jc in range(NQC):
            full[b, QCH * jc + P * jb : QCH * jc + P * jb + P] = o[jc]
    return full
